# revision 1
# baseline (speedup 1.0000x reference)
"""Sigmoid-attention block (q/k/v linear + sigmoid(q@k.T) @ v) on 8 trn2 cores.

Contract: kernel(**inputs) takes FULL unsharded numpy inputs
(query/key/value [8192, 256], Wq/Wk/Wv [256, 256], bq/bk/bv [256]) and
returns the full [8192, 256] float32 output.

Strategy: shard query rows across the 8 cores (1024 rows each); key/value
and weights replicated. Each core computes, fully on-chip:
  qT [256, 1024]  = (Wq @ queryT) + bq      (projection in transposed layout)
  kT [256, 8192]  = (Wk @ keyT) + bk
  v  [8192, 256]  = value @ Wv.T + bv       (natural layout)
  attnT[j, i]     = sigmoid(sum_h kT[h, j] qT[h, i])   j-block streamed
  outT[h, i]     += sum_j v[j, h] attnT[j, i]          accumulated in PSUM
  out             = outT.T
All matmuls run in fp32r (TF32-class precision); transposes via the PE
identity-transpose path in fp32; sigmoid+bias adds on the scalar engine.
"""
from contextlib import ExitStack

import numpy as np

import concourse.bass as bass
import concourse.mybir as mybir
import concourse.tile as tile
from concourse import bacc
from concourse.bass_utils import run_bass_kernel_spmd
from concourse.masks import make_identity

F32 = mybir.dt.float32
F32R = mybir.dt.float32r
AF = mybir.ActivationFunctionType

N_CORES = 8
N = 8192
H = 256
SLAB = N // N_CORES


def _build_attn_kernel(SLAB=SLAB, N=N, H=H):
    assert H == 256
    NJ = N // 128            # j-blocks (rows of attnT)
    NI = SLAB // 128         # i-blocks
    ICW = min(512, SLAB)     # i-chunk width for matmul moving dim
    IC = SLAB // ICW         # i-chunks
    KCW = 512                # key-chunk width (columns of kT per step)
    NKC = N // KCW
    HB = H // 128            # 2 h-blocks

    nc = bacc.Bacc()
    query = nc.dram_tensor("query", [SLAB, H], F32, kind="ExternalInput")
    key = nc.dram_tensor("key", [N, H], F32, kind="ExternalInput")
    value = nc.dram_tensor("value", [N, H], F32, kind="ExternalInput")
    Wq = nc.dram_tensor("Wq", [H, H], F32, kind="ExternalInput")
    bq = nc.dram_tensor("bq", [H], F32, kind="ExternalInput")
    Wk = nc.dram_tensor("Wk", [H, H], F32, kind="ExternalInput")
    bk = nc.dram_tensor("bk", [H], F32, kind="ExternalInput")
    Wv = nc.dram_tensor("Wv", [H, H], F32, kind="ExternalInput")
    bv = nc.dram_tensor("bv", [H], F32, kind="ExternalInput")
    out = nc.dram_tensor("out", [SLAB, H], F32, kind="ExternalOutput")

    with tile.TileContext(nc) as tc, ExitStack() as ctx:
        cpool = ctx.enter_context(tc.tile_pool(name="const", bufs=1))
        big = ctx.enter_context(tc.tile_pool(name="big", bufs=1))
        rot = ctx.enter_context(tc.tile_pool(name="rot", bufs=3))
        attnp = ctx.enter_context(tc.tile_pool(name="attnp", bufs=3))
        outp = ctx.enter_context(tc.tile_pool(name="outp", bufs=2))

        ident = cpool.tile([128, 128], F32, tag="ident")
        make_identity(nc, ident[:])

        # persistent big tensors
        kT_sb = [big.tile([128, N], F32R, tag=f"kT{hb}", name=f"kT{hb}")
                 for hb in range(HB)]
        qT_sb = [big.tile([128, SLAB], F32R, tag=f"qT{hb}", name=f"qT{hb}")
                 for hb in range(HB)]
        queryT = [big.tile([128, SLAB], F32R, tag=f"quT{hb}", name=f"quT{hb}")
                  for hb in range(HB)]
        v_t = big.tile([128, NJ, H], F32R, tag="v")  # [j-part, j-blk, h]

        # constants: biases, ones
        bq_t = [cpool.tile([128, 1], F32, tag=f"bq{hb}", name=f"bq{hb}")
                for hb in range(HB)]
        bk_t = [cpool.tile([128, 1], F32, tag=f"bk{hb}", name=f"bk{hb}")
                for hb in range(HB)]
        for hb in range(HB):
            nc.sync.dma_start(bq_t[hb][:], bq[hb * 128:(hb + 1) * 128][:, None])
            nc.sync.dma_start(bk_t[hb][:], bk[hb * 128:(hb + 1) * 128][:, None])
        bv_f = cpool.tile([1, H], F32, tag="bvf")
        nc.sync.dma_start(bv_f[:], bv[None, :])
        bv_r = cpool.tile([1, H], F32R, tag="bvr")
        nc.vector.tensor_copy(bv_r[:], bv_f[:])
        ones_f = cpool.tile([1, 128], F32, tag="onesf")
        nc.gpsimd.memset(ones_f[:], 1.0)
        ones_r = cpool.tile([1, 128], F32R, tag="onesr")
        nc.vector.tensor_copy(ones_r[:], ones_f[:])

        with (
            tc.tile_pool(name="psT", bufs=2, space="PSUM") as psT,
            tc.tile_pool(name="psP", bufs=2, space="PSUM") as psP,
            tc.tile_pool(name="psV", bufs=2, space="PSUM") as psV,
        ):
            # weights: load + PE-transpose -> wT[name][hpb] = W.T row-block
            wT = {}
            for name, wdram in (("q", Wq), ("k", Wk), ("v", Wv)):
                wT[name] = [cpool.tile([128, H], F32R, tag=f"w{name}T{b}",
                                       name=f"w{name}T{b}")
                            for b in range(HB)]
                for hb in range(HB):
                    wn = rot.tile([128, H], F32, tag="wnat")
                    nc.sync.dma_start(wn[:], wdram[hb * 128:(hb + 1) * 128, :])
                    for hpb in range(HB):
                        pt = psT.tile([128, 128], F32, tag="pt")
                        nc.tensor.transpose(
                            pt[:], wn[:, hpb * 128:(hpb + 1) * 128], ident[:]
                        )
                        nc.vector.tensor_copy(
                            wT[name][hpb][:, hb * 128:(hb + 1) * 128], pt[:]
                        )

            # queryT (transpose of query slab)
            for ib in range(NI):
                qn = rot.tile([128, H], F32, tag="xnat")
                nc.sync.dma_start(qn[:], query[ib * 128:(ib + 1) * 128, :])
                for hpb in range(HB):
                    pt = psT.tile([128, 128], F32, tag="pt")
                    nc.tensor.transpose(
                        pt[:], qn[:, hpb * 128:(hpb + 1) * 128], ident[:]
                    )
                    nc.vector.tensor_copy(
                        queryT[hpb][:, ib * 128:(ib + 1) * 128], pt[:]
                    )

            # qT projection: qT[hb] = (Wq @ queryT) + bq
            for hb in range(HB):
                for ic in range(IC):
                    pq = psP.tile([128, ICW], F32, tag="pp")
                    for hpb in range(HB):
                        nc.tensor.matmul(
                            pq[:],
                            wT["q"][hpb][:, hb * 128:(hb + 1) * 128],
                            queryT[hpb][:, ic * ICW:(ic + 1) * ICW],
                            start=(hpb == 0),
                            stop=(hpb == HB - 1),
                        )
                    nc.scalar.activation(
                        qT_sb[hb][:, ic * ICW:(ic + 1) * ICW], pq[:],
                        AF.Identity, bias=bq_t[hb][:],
                    )

            # keyT + kT projection, streamed in KCW chunks
            for c in range(NKC):
                kyT = [rot.tile([128, KCW], F32R, tag=f"kyT{hpb}",
                                name=f"kyT{hpb}")
                       for hpb in range(HB)]
                for t in range(KCW // 128):
                    kn = rot.tile([128, H], F32, tag="xnat")
                    nc.sync.dma_start(
                        kn[:], key[c * KCW + t * 128:c * KCW + (t + 1) * 128, :]
                    )
                    for hpb in range(HB):
                        pt = psT.tile([128, 128], F32, tag="pt")
                        nc.tensor.transpose(
                            pt[:], kn[:, hpb * 128:(hpb + 1) * 128], ident[:]
                        )
                        nc.vector.tensor_copy(
                            kyT[hpb][:, t * 128:(t + 1) * 128], pt[:]
                        )
                for hb in range(HB):
                    pk = psP.tile([128, KCW], F32, tag="pp")
                    for hpb in range(HB):
                        nc.tensor.matmul(
                            pk[:],
                            wT["k"][hpb][:, hb * 128:(hb + 1) * 128],
                            kyT[hpb][:],
                            start=(hpb == 0),
                            stop=(hpb == HB - 1),
                        )
                    nc.scalar.activation(
                        kT_sb[hb][:, c * KCW:(c + 1) * KCW], pk[:],
                        AF.Identity, bias=bk_t[hb][:],
                    )

            # valueT + v projection (natural layout, bias via K=1 matmul)
            for j in range(NJ):
                vn = rot.tile([128, H], F32, tag="xnat")
                nc.sync.dma_start(vn[:], value[j * 128:(j + 1) * 128, :])
                vTb = []
                for hpb in range(HB):
                    pt = psT.tile([128, 128], F32, tag="pt")
                    nc.tensor.transpose(
                        pt[:], vn[:, hpb * 128:(hpb + 1) * 128], ident[:]
                    )
                    vb = rot.tile([128, 128], F32R, tag=f"vT{hpb}",
                                  name=f"vT{hpb}")
                    nc.vector.tensor_copy(vb[:], pt[:])
                    vTb.append(vb)
                pv = psV.tile([128, H], F32, tag="pv")
                nc.tensor.matmul(pv[:], vTb[0][:], wT["v"][0][:], start=True,
                                 stop=False)
                nc.tensor.matmul(pv[:], vTb[1][:], wT["v"][1][:], start=False,
                                 stop=False)
                nc.tensor.matmul(pv[:], ones_r[:], bv_r[:], start=False,
                                 stop=True)
                nc.vector.tensor_copy(v_t[:, j, :], pv[:])

        # attention: attnT one j-block at a time, accumulate outT in PSUM
        with (
            tc.tile_pool(name="psL", bufs=2, space="PSUM") as psL,
            tc.tile_pool(name="psO", bufs=1, space="PSUM") as psO,
        ):
            ps_o = [psO.tile([128, SLAB], F32, tag=f"po{hb}", name=f"po{hb}")
                    for hb in range(HB)]
            at_tiles = [None] * NJ

            def emit_logits(j):
                pl = psL.tile([128, SLAB], F32, tag="pl")
                for hb in range(HB):
                    for ic in range(IC):
                        nc.tensor.matmul(
                            pl[:, ic * ICW:(ic + 1) * ICW],
                            kT_sb[hb][:, j * 128:(j + 1) * 128],
                            qT_sb[hb][:, ic * ICW:(ic + 1) * ICW],
                            start=(hb == 0),
                            stop=(hb == HB - 1),
                        )
                at = attnp.tile([128, SLAB], F32R, tag="at")
                nc.scalar.activation(at[:], pl[:], AF.Sigmoid)
                at_tiles[j] = at

            def emit_out_acc(j):
                at = at_tiles[j]
                for hb in range(HB):
                    for ic in range(IC):
                        nc.tensor.matmul(
                            ps_o[hb][:, ic * ICW:(ic + 1) * ICW],
                            v_t[:, j, hb * 128:(hb + 1) * 128],
                            at[:, ic * ICW:(ic + 1) * ICW],
                            start=(j == 0),
                            stop=(j == NJ - 1),
                        )
                at_tiles[j] = None

            # software-pipelined by one j so sigmoid(j) overlaps PE work
            emit_logits(0)
            for j in range(1, NJ):
                emit_logits(j)
                emit_out_acc(j - 1)
            emit_out_acc(NJ - 1)

            outT_sb = [outp.tile([128, SLAB], F32, tag=f"oT{hb}", bufs=1,
                                 name=f"oT{hb}")
                       for hb in range(HB)]
            for hb in range(HB):
                nc.vector.tensor_copy(outT_sb[hb][:], ps_o[hb][:])

        # final transpose outT -> out natural + DMA
        with tc.tile_pool(name="psF", bufs=2, space="PSUM") as psF:
            for ib in range(NI):
                ob = outp.tile([128, H], F32, tag="ob")
                for hb in range(HB):
                    pt = psF.tile([128, 128], F32, tag="ptf")
                    nc.tensor.transpose(
                        pt[:], outT_sb[hb][:, ib * 128:(ib + 1) * 128], ident[:]
                    )
                    nc.vector.tensor_copy(ob[:, hb * 128:(hb + 1) * 128], pt[:])
                nc.sync.dma_start(out[ib * 128:(ib + 1) * 128, :], ob[:])

    nc.finalize()
    return nc


_NC = None


def _get_nc():
    global _NC
    if _NC is None:
        _NC = _build_attn_kernel()
    return _NC


def _in_maps(inputs):
    full = {k: np.ascontiguousarray(np.asarray(v, dtype=np.float32))
            for k, v in inputs.items()}
    maps = []
    for c in range(N_CORES):
        m = dict(full)
        m["query"] = np.ascontiguousarray(
            full["query"][c * SLAB:(c + 1) * SLAB]
        )
        maps.append(m)
    return maps


def kernel(**inputs) -> np.ndarray:
    nc = _get_nc()
    res = run_bass_kernel_spmd(nc, _in_maps(inputs), list(range(N_CORES)))
    return np.concatenate(
        [np.asarray(res.results[c]["out"]) for c in range(N_CORES)], axis=0
    ).astype(np.float32)


# revision 3
# speedup vs baseline: 1.7724x; 1.7724x over previous
"""Sigmoid-attention block kernel for trn2 (one NeuronCore, SPMD over 8) — v2.

Math (per core, query slab of SLAB rows):
  qT [H, SLAB]   = Wq @ queryT + bq            (PE-transposed query input)
  kT [H, N]      = Wk @ keyT + bk              (streamed, PE-transposed key)
  attnT [N, SLAB] = sigmoid(kT.T @ qT)          (j-block streamed)
  out0T [H, SLAB] = sum_j value[j,:].T-blocks @ attnT[j,:]   (value UNPROJECTED)
  rowsumT [1, SLAB] = sum_j attnT[j, i]         (gpsimd partition reduce)
  outT[h,i] = sum_h' Wv[h,h'] out0T[h',i] + bv[h] * rowsumT[i]
  out  = outT.T                                 (PE transpose epilogue)

v2 vs v1: value projection folded through the attention (removes 128 PE
transposes + 192 projection matmuls + 64 copies); emission interleaves the
kv-prep stream with the attention stream one chunk apart so the PE matmul
stream stays dense and the HAM clock gate stays warm; logits PSUM is
split in 512-wide tiles so sigmoid pipelines at finer grain.
"""
from contextlib import ExitStack

import concourse.bass as bass
import concourse.bass_isa as bass_isa
import concourse.mybir as mybir
import concourse.tile as tile
from concourse import bacc
from concourse.masks import make_identity

F32 = mybir.dt.float32
F32R = mybir.dt.float32r
AF = mybir.ActivationFunctionType


def _build_attn_kernel(SLAB=1024, N=8192, H=256):
    assert H == 256
    NJ = N // 128            # j-blocks (rows of attnT)
    NI = SLAB // 128         # i-blocks
    ICW = min(512, SLAB)     # i-chunk width
    IC = SLAB // ICW
    KCW = 512                # key-chunk width = 4 j-blocks
    NKC = N // KCW
    JPC = KCW // 128         # j-blocks per chunk (4)
    HB = H // 128            # 2

    nc = bacc.Bacc()
    query = nc.dram_tensor("query", [SLAB, H], F32, kind="ExternalInput")
    key = nc.dram_tensor("key", [N, H], F32, kind="ExternalInput")
    value = nc.dram_tensor("value", [N, H], F32, kind="ExternalInput")
    Wq = nc.dram_tensor("Wq", [H, H], F32, kind="ExternalInput")
    bq = nc.dram_tensor("bq", [H], F32, kind="ExternalInput")
    Wk = nc.dram_tensor("Wk", [H, H], F32, kind="ExternalInput")
    bk = nc.dram_tensor("bk", [H], F32, kind="ExternalInput")
    Wv = nc.dram_tensor("Wv", [H, H], F32, kind="ExternalInput")
    bv = nc.dram_tensor("bv", [H], F32, kind="ExternalInput")
    out = nc.dram_tensor("out", [SLAB, H], F32, kind="ExternalOutput")

    with tile.TileContext(nc) as tc, ExitStack() as ctx:
        cpool = ctx.enter_context(tc.tile_pool(name="const", bufs=1))
        big = ctx.enter_context(tc.tile_pool(name="big", bufs=1))
        rot = ctx.enter_context(tc.tile_pool(name="rot", bufs=4))
        valp = ctx.enter_context(tc.tile_pool(name="valp", bufs=12))
        attnp = ctx.enter_context(tc.tile_pool(name="attnp", bufs=3))
        redp = ctx.enter_context(tc.tile_pool(name="redp", bufs=2))
        outp = ctx.enter_context(tc.tile_pool(name="outp", bufs=2))

        ident = cpool.tile([128, 128], F32, tag="ident")
        make_identity(nc, ident[:])

        kT_sb = [big.tile([128, N], F32R, tag=f"kT{hb}", name=f"kT{hb}")
                 for hb in range(HB)]
        qT_sb = [big.tile([128, SLAB], F32R, tag=f"qT{hb}", name=f"qT{hb}")
                 for hb in range(HB)]
        queryT = [big.tile([128, SLAB], F32R, tag=f"quT{hb}", name=f"quT{hb}")
                  for hb in range(HB)]

        # biases
        bq_t = [cpool.tile([128, 1], F32, tag=f"bq{hb}", name=f"bq{hb}")
                for hb in range(HB)]
        bk_t = [cpool.tile([128, 1], F32, tag=f"bk{hb}", name=f"bk{hb}")
                for hb in range(HB)]
        for hb in range(HB):
            nc.sync.dma_start(bq_t[hb][:], bq[hb * 128:(hb + 1) * 128][:, None])
            nc.sync.dma_start(bk_t[hb][:], bk[hb * 128:(hb + 1) * 128][:, None])
        bv_f = cpool.tile([1, H], F32, tag="bvf")
        nc.sync.dma_start(bv_f[:], bv[None, :])
        bv_r = cpool.tile([1, H], F32R, tag="bvr")
        nc.vector.tensor_copy(bv_r[:], bv_f[:])

        # rowsum accumulator (f32, accumulated via SWDGE dma accumulate)
        rowsum = cpool.tile([1, SLAB], F32, tag="rowsum")
        nc.gpsimd.memset(rowsum[:], 0.0)

        psA = ctx.enter_context(tc.tile_pool(name="psA", bufs=2, space="PSUM"))
        psL = ctx.enter_context(tc.tile_pool(name="psL", bufs=2, space="PSUM"))
        psO = ctx.enter_context(tc.tile_pool(name="psO", bufs=1, space="PSUM"))

        # ---- phase A: weights, queryT, qT ----
        wT = {}
        for name, wdram in (("q", Wq), ("k", Wk), ("v", Wv)):
            wT[name] = [cpool.tile([128, H], F32R, tag=f"w{name}T{b}",
                                   name=f"w{name}T{b}")
                        for b in range(HB)]
            for hb in range(HB):
                wn = rot.tile([128, H], F32, tag="wnat")
                nc.sync.dma_start(wn[:], wdram[hb * 128:(hb + 1) * 128, :])
                for hpb in range(HB):
                    pt = psA.tile([128, 512], F32, tag="ps")
                    nc.tensor.transpose(
                        pt[:, :128], wn[:, hpb * 128:(hpb + 1) * 128], ident[:]
                    )
                    nc.vector.tensor_copy(
                        wT[name][hpb][:, hb * 128:(hb + 1) * 128], pt[:, :128]
                    )

        for ib in range(NI):
            qn = rot.tile([128, H], F32, tag="xnat")
            nc.sync.dma_start(qn[:], query[ib * 128:(ib + 1) * 128, :])
            for hpb in range(HB):
                pt = psA.tile([128, 512], F32, tag="ps")
                nc.tensor.transpose(
                    pt[:, :128], qn[:, hpb * 128:(hpb + 1) * 128], ident[:]
                )
                nc.vector.tensor_copy(
                    queryT[hpb][:, ib * 128:(ib + 1) * 128], pt[:, :128]
                )

        for hb in range(HB):
            for ic in range(IC):
                pq = psA.tile([128, 512], F32, tag="ps")
                for hpb in range(HB):
                    nc.tensor.matmul(
                        pq[:, :ICW],
                        wT["q"][hpb][:, hb * 128:(hb + 1) * 128],
                        queryT[hpb][:, ic * ICW:(ic + 1) * ICW],
                        start=(hpb == 0),
                        stop=(hpb == HB - 1),
                    )
                nc.scalar.activation(
                    qT_sb[hb][:, ic * ICW:(ic + 1) * ICW], pq[:, :ICW],
                    AF.Identity, bias=bq_t[hb][:],
                )

        # ---- phase B: chunk-pipelined kv prep + attention ----
        val_r = [None] * NJ   # rotating f32r copies of value j-blocks
        at_tiles = [None] * NJ
        ps_o = [psO.tile([128, SLAB], F32, tag=f"po{hb}", name=f"po{hb}")
                for hb in range(HB)]

        def emit_kv_prep(c, t, kyT):
            """Load key/value j-block j=c*JPC+t; transpose key into kyT."""
            j = c * JPC + t
            kn = rot.tile([128, H], F32, tag="xnat")
            nc.sync.dma_start(kn[:], key[j * 128:(j + 1) * 128, :])
            for hpb in range(HB):
                pt = psA.tile([128, 512], F32, tag="ps")
                nc.tensor.transpose(
                    pt[:, :128], kn[:, hpb * 128:(hpb + 1) * 128], ident[:]
                )
                nc.vector.tensor_copy(kyT[hpb][:, t * 128:(t + 1) * 128],
                                      pt[:, :128])
            vn = rot.tile([128, H], F32, tag="xnat")
            nc.sync.dma_start(vn[:], value[j * 128:(j + 1) * 128, :])
            vr = valp.tile([128, H], F32R, tag="valr")
            nc.vector.tensor_copy(vr[:], vn[:])
            val_r[j] = vr

        def emit_kT_proj(c, kyT):
            for hb in range(HB):
                pk = psA.tile([128, 512], F32, tag="ps")
                for hpb in range(HB):
                    nc.tensor.matmul(
                        pk[:],
                        wT["k"][hpb][:, hb * 128:(hb + 1) * 128],
                        kyT[hpb][:],
                        start=(hpb == 0),
                        stop=(hpb == HB - 1),
                    )
                nc.scalar.activation(
                    kT_sb[hb][:, c * KCW:(c + 1) * KCW], pk[:],
                    AF.Identity, bias=bk_t[hb][:],
                )

        def emit_logits(j):
            at = attnp.tile([128, SLAB], F32R, tag="at")
            for ic in range(IC):
                pl = psL.tile([128, ICW], F32, tag="pl")
                for hb in range(HB):
                    nc.tensor.matmul(
                        pl[:],
                        kT_sb[hb][:, j * 128:(j + 1) * 128],
                        qT_sb[hb][:, ic * ICW:(ic + 1) * ICW],
                        start=(hb == 0),
                        stop=(hb == HB - 1),
                    )
                nc.scalar.activation(at[:, ic * ICW:(ic + 1) * ICW], pl[:],
                                     AF.Sigmoid)
            at_tiles[j] = at
            # rowsum contribution on gpsimd (idle engine) + dma accumulate
            red = redp.tile([128, SLAB], F32, tag="red")
            nc.gpsimd.partition_all_reduce(
                red[:], at[:].bitcast(F32), channels=128,
                reduce_op=bass_isa.ReduceOp.add,
            )
            nc.gpsimd.dma_start(out=rowsum[:], in_=red[0:1, :],
                                accum_op=mybir.AluOpType.add)

        def emit_out_acc(j):
            at = at_tiles[j]
            for hb in range(HB):
                for ic in range(IC):
                    nc.tensor.matmul(
                        ps_o[hb][:, ic * ICW:(ic + 1) * ICW],
                        val_r[j][:, hb * 128:(hb + 1) * 128],
                        at[:, ic * ICW:(ic + 1) * ICW],
                        start=(j == 0),
                        stop=(j == NJ - 1),
                    )
            at_tiles[j] = None
            val_r[j] = None

        for c in range(NKC):
            kyT = [rot.tile([128, KCW], F32R, tag=f"kyT{hpb}", name=f"kyT{hpb}",
                            bufs=2)
                   for hpb in range(HB)]
            for t in range(JPC):
                emit_kv_prep(c, t, kyT)
                if c >= 1:
                    j_att = (c - 1) * JPC + t
                    emit_logits(j_att)
                    if j_att >= 1:
                        emit_out_acc(j_att - 1)
            emit_kT_proj(c, kyT)

        # drain: attention for the last chunk
        for t in range(JPC):
            j_att = (NKC - 1) * JPC + t
            emit_logits(j_att)
            emit_out_acc(j_att - 1)
        emit_out_acc(NJ - 1)

        # ---- epilogue: outT = Wv @ out0T + bv (x) rowsum; out = outT.T ----
        out0T = [outp.tile([128, SLAB], F32R, tag=f"o0T{hb}", bufs=1,
                           name=f"o0T{hb}")
                 for hb in range(HB)]
        for hb in range(HB):
            nc.vector.tensor_copy(out0T[hb][:], ps_o[hb][:])
        rowsum_r = cpool.tile([1, SLAB], F32R, tag="rowsumr")
        nc.vector.tensor_copy(rowsum_r[:], rowsum[:])

        outT_sb = [outp.tile([128, SLAB], F32, tag=f"oT{hb}", bufs=1,
                             name=f"oT{hb}")
                   for hb in range(HB)]
        for hb in range(HB):
            for ic in range(IC):
                pf = psL.tile([128, ICW], F32, tag="pl")
                for hpb in range(HB):
                    nc.tensor.matmul(
                        pf[:],
                        wT["v"][hpb][:, hb * 128:(hb + 1) * 128],
                        out0T[hpb][:, ic * ICW:(ic + 1) * ICW],
                        start=(hpb == 0),
                        stop=False,
                    )
                nc.tensor.matmul(
                    pf[:],
                    bv_r[:, hb * 128:(hb + 1) * 128],
                    rowsum_r[:, ic * ICW:(ic + 1) * ICW],
                    start=False,
                    stop=True,
                )
                nc.vector.tensor_copy(
                    outT_sb[hb][:, ic * ICW:(ic + 1) * ICW], pf[:]
                )

        for ib in range(NI):
            ob = outp.tile([128, H], F32, tag="ob")
            for hb in range(HB):
                pt = psA.tile([128, 512], F32, tag="ps")
                nc.tensor.transpose(
                    pt[:, :128], outT_sb[hb][:, ib * 128:(ib + 1) * 128],
                    ident[:]
                )
                nc.vector.tensor_copy(ob[:, hb * 128:(hb + 1) * 128],
                                      pt[:, :128])
            nc.sync.dma_start(out[ib * 128:(ib + 1) * 128, :], ob[:])

    nc.finalize()
    return nc


import numpy as np
from concourse.bass_utils import run_bass_kernel_spmd

N_CORES = 8
N_FULL = 8192
H_FULL = 256
SLAB_FULL = N_FULL // N_CORES

_NC = None


def _get_nc():
    global _NC
    if _NC is None:
        _NC = _build_attn_kernel(SLAB=SLAB_FULL, N=N_FULL, H=H_FULL)
    return _NC


def _in_maps(inputs):
    full = {k: np.ascontiguousarray(np.asarray(v, dtype=np.float32))
            for k, v in inputs.items()}
    # fold the v-projection bias through Wv: delta = Wv^-1 bv, added to value
    # rows on-chip (attn @ (value + 1(x)delta) @ Wv.T == attn@v + rowsum(x)bv)
    Wv64 = full["Wv"].astype(np.float64)
    bv64 = full["bv"].astype(np.float64)
    try:
        delta = np.linalg.solve(Wv64, bv64)
    except np.linalg.LinAlgError:
        delta = np.linalg.lstsq(Wv64, bv64, rcond=None)[0]
    delta_bc = np.ascontiguousarray(
        np.tile(delta.astype(np.float32)[None, :], (128, 1))
    )
    maps = []
    for c in range(N_CORES):
        m = {
            "query": np.ascontiguousarray(
                full["query"][c * SLAB_FULL:(c + 1) * SLAB_FULL]
            ),
            "key": full["key"],
            "value": full["value"],
            "Wq": full["Wq"],
            "bq": full["bq"],
            "Wk": full["Wk"],
            "bk": full["bk"],
            "Wv": full["Wv"],
            "delta_bc": delta_bc,
        }
        maps.append(m)
    return maps


def kernel(**inputs) -> np.ndarray:
    nc = _get_nc()
    res = run_bass_kernel_spmd(nc, _in_maps(inputs), list(range(N_CORES)))
    return np.concatenate(
        [np.asarray(res.results[c]["out"]) for c in range(N_CORES)], axis=0
    ).astype(np.float32)
